# revision 1
# baseline (speedup 1.0000x reference)
"""AdaConv Trainium2 kernel — 8-core SPMD, data-parallel over batch.

v5: fully local per-core computation — NO collective. Each core receives
only ITS sample's style/image and the FULL (replicated) prediction-net
weights, and computes its own dynamic conv weights locally. The AllToAll
of v3/v4 cost ~40us in CC mesh setup; replicating w2/pk2/pb2 (+9.5MiB
fp16 DMA, hidden under compute) and spending ~19us extra PE time on the
unsliced layer-2 matmuls removes it entirely and kills all cross-core
sync variance.

Structure:
  * All dynamic tensors fp16 (full PE rate), fp16 output upcast on host.
  * Weights/constants host-packed into few [128, W] walls; w2 loaded in
    4 chunk-quarters so stage C chunk 0 starts while later quarters
    stream. dma_start costs ~650ns serial issue time per engine, so
    issues are few and spread over SP/scalar/gpsimd queues.
  * Stage A (own sample): layer-1 h via 16 N=16 matmuls, lrelu fused in
    windowed psum drains (im2col layout for dw2); dw2 weights-as-moving
    (16 N=512 MMs per chunk-quarter, psum [9,512]); pooled path via
    M=1 stationaries. Static layer-2 biases folded receiver-side.
  * Dynamic weights roundtrip through tiny DRAM scratch so the
    (channel, tap) deinterleave is a single 3-D DMA gather per chunk.
  * Stage B: block-diag fp16 stationaries S[ch] via perm-matmuls +
    per-partition scales + select-matmuls + mask.
  * Stage C: grouped 3x3 conv, 9 psum-accumulated fp16 matmuls per
    2-sub wave, 4 rotating psum banks; per-half-chunk output stores.
  * Software pipeline: dw2(q)/B(q) run one chunk ahead of C(q).
"""
import sys
import types

sys.path.insert(0, "/opt/trn_rl_repo")

import numpy as np

import concourse.bass as bass
import concourse.mybir as mybir

N = 8          # batch == cores
CIN = 512
COUT = 512
HW = 64        # spatial
HWP = 66       # padded
XPW = 4384     # per-chunk padded width (66*66=4356 used)
NPOS = 16      # style spatial 4x4

# wallA layout (cols): st-own | w1 | b1t
CA_ST = 0
CA_W1 = CA_ST + NPOS * 4
CA_B1 = CA_W1 + 2048
WA = CA_B1 + 4            # 2116
# wallP1: pooled layer-1 weights
CP_PW1 = 0
CP_PWB1 = CP_PW1 + 4096
WP1 = CP_PWB1 + 8         # 4104
# wmq: pooled layer-2 moving quarters [128, 4*(pk 512 | pb 128)]
# wallP3: stage-B constants
C2_PERM = 0
C2_IDENT = C2_PERM + 512
C2_SEL = C2_IDENT + 128
C2_MASK = C2_SEL + 1152
C2_BIASD = C2_MASK + 384
C2_BIASPK = C2_BIASD + 144
C2_BIASPB = C2_BIASPK + 16
WP3 = C2_BIASPB + 4       # 2340

F32 = mybir.dt.float32
F16 = mybir.dt.float16
F8 = mybir.dt.float8e4


# ---------------------------------------------------------------- tile patch
def _install_tile_patch():
    """walrus here rejects Drain instructions with >1 sync-wait; spread the
    Tile tail-drain waits over individual SP nops."""
    import concourse.tile as tile_mod
    from concourse.vector_clock import ScopedClock

    def _patched(self, tick_clock, wait_clock):
        nc = self.nc
        drain_inst = nc.sync.drain()
        wait_clock.add_sem_waits(
            drain_inst.ins, ScopedClock({None: tick_clock.global_clock})
        )
        waits = list(drain_inst.ins.sync_info.on_wait or [])
        if len(waits) > 1:
            drain_inst.ins.sync_info.on_wait = waits[:1]
            for w in waits[1:]:
                nop = nc.sync.nop(nofuse=True, hint="tail_wait_split")
                if nop.ins.sync_info is None:
                    nop.ins.sync_info = mybir.SyncInfo(on_wait=[w], on_update=[])
                else:
                    nop.ins.sync_info.on_wait = [w]
        nc.all_engine_barrier()
        assert self.sems is not None
        popped = nc._tile_sem_poison_stack.pop()
        assert popped is self._sem_poison
        nc.clear_and_free_semaphores(list(self.sems.allocated().values()))
        nc.all_engine_barrier()

    tile_mod.TileContext._drain_and_barrier = _patched


_install_tile_patch()
from concourse.tile import TileContext  # noqa: E402


def install_profile_shim():
    """antenv.axon_hooks is missing from this image; recreate it so
    run_bass_kernel_spmd(trace=True) can capture NTFF profiles."""
    if "antenv.axon_hooks" in sys.modules:
        return
    import antenv

    mod = types.ModuleType("antenv.axon_hooks")
    mod._hook = None
    mod.set_axon_ntff_profile_hook = lambda h: setattr(mod, "_hook", h)
    mod.get_axon_ntff_profile_hook = lambda: mod._hook
    sys.modules["antenv.axon_hooks"] = mod
    antenv.axon_hooks = mod
    try:
        if "/root/.axon_site" not in sys.path:
            sys.path.insert(0, "/root/.axon_site")
        from trn_agent_boot.trn_boot import _ntff_profile_via_ctypes

        hook = _ntff_profile_via_ctypes("/opt/axon/libaxon_pjrt.so")
        mod.set_axon_ntff_profile_hook(hook)
    except Exception:
        pass


def _ap(t_ap, offset, dims):
    """Custom flat AP over a tile's underlying tensor."""
    return bass.AP(t_ap.tensor, offset, [list(d) for d in dims])


def _pt(t):
    """Physical partition pitch (elements) of a tile."""
    return t[:, :].ap[0][0]


def _split_excess_waits(nc, max_waits=1):
    """This walrus build rejects instructions carrying more than ~1 sync-wait.
    Move excess waits onto same-engine NoOps inserted just before."""
    n_split = 0
    for f in nc.m.functions:
        for bb in f.blocks:
            newlist = []
            for inst in bb.instructions:
                si = getattr(inst, "sync_info", None)
                if si is not None and si.on_wait and len(si.on_wait) > max_waits:
                    waits = list(si.on_wait)
                    for k, w in enumerate(waits[max_waits:]):
                        nop = mybir.InstNoOp(
                            name=f"{inst.name}_ws{k}",
                            engine=inst.engine,
                            bass_nofuse=True,
                            sync_info=mybir.SyncInfo(on_wait=[w], on_update=[]),
                        )
                        newlist.append(nop)
                        n_split += 1
                    si.on_wait = waits[:max_waits]
                newlist.append(inst)
            try:
                bb.instructions[:] = newlist
            except TypeError:
                bb.set_instructions(newlist)
    return n_split


LRELU = mybir.ActivationFunctionType.Lrelu
IDENT = mybir.ActivationFunctionType.Identity
COPY = mybir.ActivationFunctionType.Copy


def build_nc():
    nc = bass.Bass(target_bir_lowering=False)

    wallA = nc.declare_dram_parameter("wallA", [128, WA], F16, isOutput=False)
    wallP1 = nc.declare_dram_parameter("wallP1", [128, WP1], F16, isOutput=False)
    wmq = [nc.declare_dram_parameter(f"wmq{q}", [128, 4 * 640], F16,
                                     isOutput=False) for q in range(4)]
    wallP3 = nc.declare_dram_parameter("wallP3", [128, WP3], F16, isOutput=False)
    w2q = [nc.declare_dram_parameter(f"w2q{q}", [128, 16 * 512], F16,
                                     isOutput=False) for q in range(4)]
    xp4 = [nc.declare_dram_parameter(f"xp{ch}", [128, XPW], F16,
                                 isOutput=False) for ch in range(4)]
    out = nc.declare_dram_parameter("out", [COUT, HW * HW], F16, isOutput=True)

    with TileContext(nc) as tc:
        with (
            tc.tile_pool(name="sb", bufs=1) as sb,
            tc.tile_pool(name="sbx", bufs=1) as sbx,
            tc.tile_pool(name="sbo", bufs=2) as sbo,
            tc.tile_pool(name="psb", bufs=2, space="PSUM") as psb,
            tc.tile_pool(name="psc", bufs=4, space="PSUM") as psc,
            tc.tile_pool(name="dram", bufs=1, space="DRAM") as dram,
        ):
            wa = sb.tile([128, WA], F16, tag="wa", name="wa")
            nc.sync.dma_start(out=wa[:, :], in_=wallA[:, :])
            wp = sb.tile([128, WP1], F16, tag="wp", name="wp")
            nc.sync.dma_start(out=wp[:, :], in_=wallP1[:, :])
            w2sb = [sb.tile([128, 16 * 512], F16, tag=f"w2sb{q}",
                            name=f"w2sb{q}") for q in range(4)]
            nc.sync.dma_start(out=w2sb[0][:, :], in_=w2q[0][:, :])
            wms = [sb.tile([128, 4 * 640], F16, tag=f"wms{q}",
                           name=f"wms{q}") for q in range(4)]
            nc.sync.dma_start(out=wms[0][:, :], in_=wmq[0][:, :])
            wc = sb.tile([128, WP3], F16, tag="wc", name="wc")
            nc.sync.dma_start(out=wc[:, :], in_=wallP3[:, :])
            xps = [sbx.tile([128, XPW], F16, tag=f"xps{ch}", name=f"xps{ch}")
                   for ch in range(4)]
            nc.sync.dma_start(out=xps[0][:, :], in_=xp4[0][:, :])
            nc.sync.dma_start(out=w2sb[1][:, :], in_=w2q[1][:, :])
            nc.sync.dma_start(out=wms[1][:, :], in_=wmq[1][:, :])
            nc.sync.dma_start(out=xps[1][:, :], in_=xp4[1][:, :])
            nc.sync.dma_start(out=w2sb[2][:, :], in_=w2q[2][:, :])
            nc.sync.dma_start(out=wms[2][:, :], in_=wmq[2][:, :])
            nc.sync.dma_start(out=xps[2][:, :], in_=xp4[2][:, :])
            nc.sync.dma_start(out=w2sb[3][:, :], in_=w2q[3][:, :])
            nc.sync.dma_start(out=wms[3][:, :], in_=wmq[3][:, :])
            nc.sync.dma_start(out=xps[3][:, :], in_=xp4[3][:, :])
            wap, wpp, wcp = _pt(wa), _pt(wp), _pt(wc)

            # PE warmup during the initial load window: ~10 dummy matmuls
            # release the HAM clock throttle before real work arrives.
            wu = sb.tile([128, 512], F16, tag="wu", name="wu")
            nc.vector.memset(wu[:, :], 0.0)
            for _ in range(10):
                pw_ = psc.tile([128, 512], F32, tag="pc", name="pw_")
                nc.tensor.matmul(
                    pw_[:, :], wu[:, 0:128], wu[:, :],
                    start=True, stop=True,
                )

            def wA(col, np_, nf):
                return _ap(wa, col, [[wap, np_], [1, nf]])

            def wP(col, np_, nf):
                return _ap(wp, col, [[wpp, np_], [1, nf]])

            def wC(col, np_, nf):
                return _ap(wc, col, [[wcp, np_], [1, nf]])

            # ------------ stage A: h = lrelu(W1 s + b1) for OWN sample,
            # drained into im2col h2[ot][:, dydx*9:+9] = (ty, tx) windows
            h2 = [sb.tile([128, 36], F16, tag=f"h2{ot}", name=f"h2{ot}")
                  for ot in range(4)]
            for ot in range(4):
                pa = psb.tile([128, NPOS], F32, tag="sA", name="pa")
                for it in range(4):
                    nc.tensor.matmul(
                        pa[:, :],
                        wA(CA_W1 + it * CIN + ot * 128, 128, 128),
                        wA(CA_ST + it * NPOS, 128, NPOS),
                        start=(it == 0),
                        stop=(it == 3),
                    )
                pap = _pt(pa)
                for dy in range(2):
                    for dx in range(2):
                        nc.scalar.activation(
                            h2[ot][:, (dy * 2 + dx) * 9:(dy * 2 + dx + 1) * 9],
                            _ap(pa, dy * 4 + dx, [[pap, 128], [4, 3], [1, 3]]),
                            LRELU,
                            bias=wA(CA_B1 + ot, 128, 1), alpha=0.01,
                        )

            # ------------ stage A: pooled path, own sample (width-1)
            pks = [dram.tile([640], F16, name=f"pks{q}") for q in range(4)]

            def pooled1():
             sp = [sb.tile([128, 1], F16, tag=f"sp{i}", name=f"sp{i}")
                   for i in range(4)]
             with nc.allow_low_precision("16-term style pool in fp16"):
                 for i in range(4):
                     nc.vector.tensor_reduce(
                         sp[i][:, :],
                         _ap(wa, CA_ST + i * NPOS, [[wap, 128], [1, NPOS]]),
                         axis=mybir.AxisListType.X,
                         op=mybir.AluOpType.add,
                     )
             ac = []
             for po in range(8):
                 pp = psb.tile([128, 1], F32, tag="sA", name="pp")
                 for it in range(4):
                     nc.tensor.matmul(
                         pp[:, :],
                         wP(CP_PW1 + it * 2 * CIN + po * 128, 128, 128),
                         sp[it][:, :],
                         start=(it == 0),
                         stop=(it == 3),
                     )
                 a = sb.tile([128, 1], F16, tag=f"ac{po}", name=f"ac{po}")
                 nc.scalar.activation(
                     a[:, :], pp[:, :], LRELU,
                     bias=wP(CP_PWB1 + po, 128, 1), alpha=0.01,
                 )
                 ac.append(a)
             return ac

            def pooled2_q(ac, q):
             # pk2|pb2 own-sample quarter: psum [1,512](pk) + [1,128](pb)
             wmpq = _pt(wms[q])
             pkbo = sb.tile([1, 640], F16, tag=f"pkbo{q}", name=f"pkbo{q}")
             for seg in range(2):
                 nf = 512 if seg == 0 else 128
                 pko = psb.tile([1, 512], F32, tag="sA", name="pko")
                 for it in range(4):
                     nc.tensor.matmul(
                         pko[:, 0:nf],
                         ac[it if seg == 0 else 4 + it][:, :],
                         _ap(wms[q], it * 640 + seg * 512,
                             [[wmpq, 128], [1, nf]]),
                         start=(it == 0),
                         stop=(it == 3),
                     )
                 nc.scalar.activation(
                     pkbo[:, seg * 512:seg * 512 + nf], pko[:, 0:nf], COPY)
             nc.scalar.dma_start(
                 out=_ap(pks[q][:], 0, [[1, 640]]), in_=pkbo[:, :])

            # ------------ dw2 per chunk-quarter (weights-as-moving),
            # psum [9=(ty,tx), 512=k-slice] -> DRAM scratch
            dws = [dram.tile([9 * 512], F16, name=f"dws{q}")
                   for q in range(4)]

            def dw2_q(q):
                pd = psb.tile([9, 512], F32, tag="sA", name="pd")
                k = 0
                for ib in range(4):
                    for dydx in range(4):
                        nc.tensor.matmul(
                            pd[:, :],
                            h2[ib][:, dydx * 9:(dydx + 1) * 9],
                            _ap(w2sb[q], (ib * 4 + dydx) * 512,
                                [[16 * 512, 128], [1, 512]]),
                            start=(k == 0),
                            stop=(k == 15),
                        )
                        k += 1
                dwo = sb.tile([9, 512], F16, tag="dwo", name="dwo")
                nc.scalar.activation(dwo[:, :], pd[:, :], COPY)
                nc.scalar.dma_start(
                    out=_ap(dws[q][:], 0, [[512, 9], [1, 512]]),
                    in_=dwo[:, :],
                )

            # ------------ stage B + stage C
            S = [sb.tile([128, 9 * 128], F16, tag=f"S{ch}", name=f"S{ch}")
                 for ch in range(4)]
            PBf = [None] * 4

            def stage_b(ch):
                # gathers from local scratch: D on SP, PK scalar, PB gpsimd
                D = sb.tile([128, 40], F16, tag=f"D{ch}", name=f"D{ch}")
                PKr = sb.tile([128, 8], F16, tag=f"PKr{ch}", name=f"PKr{ch}")
                PBr = sb.tile([128, 8], F16, tag=f"PBr{ch}", name=f"PBr{ch}")
                dpt = _pt(D)
                nc.sync.dma_start(
                    out=_ap(D, 0, [[dpt, 128], [4, 9], [1, 4]]),
                    in_=_ap(dws[ch][:], 0, [[4, 128], [512, 9], [1, 4]]),
                )
                nc.scalar.dma_start(
                    out=_ap(PKr, 0, [[_pt(PKr), 128], [1, 4]]),
                    in_=_ap(pks[ch][:], 0, [[4, 128], [1, 4]]),
                )
                nc.scalar.dma_start(
                    out=_ap(PBr, 0, [[_pt(PBr), 128], [1, 1]]),
                    in_=_ap(pks[ch][:], 512, [[1, 128], [1, 1]]),
                )
                # receiver-side static biases
                nc.vector.tensor_tensor(
                    D[:, 0:36], D[:, 0:36],
                    wC(C2_BIASD + ch * 36, 128, 36),
                    op=mybir.AluOpType.add,
                )
                PKb = sb.tile([128, 4], F32, tag=f"PKb{ch}", name=f"PKb{ch}")
                nc.vector.tensor_tensor(
                    PKb[:, :], PKr[:, 0:4],
                    wC(C2_BIASPK + ch * 4, 128, 4),
                    op=mybir.AluOpType.add,
                )
                pbf = sb.tile([128, 1], F32, tag=f"PBf{ch}", name=f"PBf{ch}")
                nc.vector.tensor_tensor(
                    pbf[:, :], PBr[:, 0:1],
                    wC(C2_BIASPB + ch, 128, 1),
                    op=mybir.AluOpType.add,
                )
                PBf[ch] = pbf
                # W_eff = sum_m PK[:,m] * (perm_m @ D)
                dp = psb.tile([128, 144], F32, tag="sB", name="dp")
                for m2 in range(4):
                    nc.tensor.matmul(
                        dp[:, m2 * 36:(m2 + 1) * 36],
                        wC(C2_PERM + m2 * 128, 128, 128),
                        D[:, 0:36],
                        start=True,
                        stop=True,
                    )
                wef = sb.tile([128, 36], F16, tag=f"wef{ch}", name=f"wef{ch}")
                tmp = sb.tile([128, 36], F16, tag=f"wtm{ch}", name=f"wtm{ch}")
                nc.vector.tensor_scalar_mul(wef[:, :], dp[:, 0:36], PKb[:, 0:1])
                for m2 in range(1, 4):
                    nc.vector.tensor_scalar_mul(
                        tmp[:, :], dp[:, m2 * 36:(m2 + 1) * 36], PKb[:, m2:m2 + 1]
                    )
                    nc.vector.tensor_add(wef[:, :], wef[:, :], tmp[:, :])
                # expand W_eff -> block-diag S via transpose + select-matmuls
                tpp = psb.tile([36, 128], F16, tag="sB", name="tpp")
                nc.tensor.matmul(
                    tpp[:, :], wef[:, :], wC(C2_IDENT, 128, 128),
                    is_transpose=True, start=True, stop=True,
                )
                wefT = sb.tile([36, 128], F16, tag=f"wefT{ch}", name=f"wefT{ch}")
                nc.vector.tensor_copy(wefT[:, :], tpp[:, :])
                for grp in range(3):
                    sps = psb.tile([128, 3 * 128], F32, tag="sB", name="sps")
                    for tt in range(3):
                        t = grp * 3 + tt
                        nc.tensor.matmul(
                            sps[:, tt * 128:(tt + 1) * 128],
                            wC(C2_SEL + t * 128, 36, 128),
                            wefT[:, :],
                            start=True, stop=True,
                        )
                    nc.vector.tensor_tensor(
                        S[ch][:, grp * 384:(grp + 1) * 384], sps[:, :],
                        wC(C2_MASK, 128, 384),
                        op=mybir.AluOpType.mult,
                    )

            def stage_c(ch, prelude=None):
                osb = sbo.tile([128, HW * HW], F16, tag="osb", name="osb")
                for wave in range(4):
                    if wave == 1 and prelude is not None:
                        prelude()
                    pcs = [psc.tile([128, 512], F32, tag="pc", name="pc")
                           for _ in range(2)]
                    for tap in range(9):
                        di, dj = tap // 3, tap % 3
                        lhs = S[ch][:, tap * 128:(tap + 1) * 128]
                        for kk, pct in enumerate(pcs):
                            r0 = (wave * 2 + kk) * 8
                            rhs = _ap(xps[ch], (r0 + di) * HWP + dj,
                                      [[XPW, 128], [HWP, 8], [1, HW]])
                            nc.tensor.matmul(
                                pct[:, :],
                                lhs,
                                rhs,
                                start=(tap == 0),
                                stop=(tap == 8),
                            )
                    for kk, pct in enumerate(pcs):
                        s8 = wave * 2 + kk
                        nc.scalar.activation(
                            osb[:, s8 * 512:(s8 + 1) * 512], pct[:, :], IDENT,
                            bias=PBf[ch][:, 0:1],
                        )
                    if wave % 2 == 1:
                        h0 = (wave - 1) * 1024
                        nc.sync.dma_start(
                            out=out[ch * 128:(ch + 1) * 128, h0:h0 + 2048],
                            in_=osb[:, h0:h0 + 2048],
                        )

            acs = pooled1()
            dw2_q(0)
            pooled2_q(acs, 0)
            stage_b(0)

            def mk_prelude(q):
                def f():
                    dw2_q(q)
                    pooled2_q(acs, q)
                    stage_b(q)
                return f

            stage_c(0, mk_prelude(1))
            stage_c(1, mk_prelude(2))
            stage_c(2, mk_prelude(3))
            stage_c(3)

    _split_excess_waits(nc)
    return nc


_NC_CACHE = {}


def _get_nc():
    if "nc" not in _NC_CACHE:
        _NC_CACHE["nc"] = build_nc()
    return _NC_CACHE["nc"]


def _pack128(arr):
    """[512, X] -> [128, 4*X] with free idx = blk*X + x."""
    xw = arr.shape[1]
    return np.ascontiguousarray(
        arr.reshape(4, 128, xw).transpose(1, 0, 2).reshape(128, 4 * xw))


def make_in_maps(inputs):
    """Host-side shard/layout prep (cast + layout only)."""
    f16 = np.float16
    style = np.asarray(inputs["style_encoding"], np.float32)
    pred = np.asarray(inputs["predicted"], np.float32)
    w1 = np.asarray(inputs["dw1_w"], np.float32).reshape(512, 512)
    w2 = np.asarray(inputs["dw2_w"], np.float32).reshape(2048, 512, 2, 2)
    pk1 = np.asarray(inputs["pk1_w"], np.float32).reshape(512, 512)
    pk2 = np.asarray(inputs["pk2_w"], np.float32).reshape(2048, 512)
    pb1 = np.asarray(inputs["pb1_w"], np.float32).reshape(512, 512)
    pb2 = np.asarray(inputs["pb2_w"], np.float32).reshape(512, 512)
    b1 = np.asarray(inputs["dw1_b"], np.float32)
    b2 = np.asarray(inputs["dw2_b"], np.float32)
    bk1 = np.asarray(inputs["pk1_b"], np.float32)
    bk2 = np.asarray(inputs["pk2_b"], np.float32)
    bb1 = np.asarray(inputs["pb1_b"], np.float32)
    bb2 = np.asarray(inputs["pb2_b"], np.float32)

    # ---- shared walls
    w1p = _pack128(np.ascontiguousarray(w1.T))
    b1t = b1.reshape(4, 128).T
    # fold the 1/16 spatial mean into the first pooled layer's weights
    pw1p = _pack128(np.ascontiguousarray(
        np.concatenate([pk1.T, pb1.T], axis=1) * (1.0 / NPOS)))
    pwb1 = np.concatenate(
        [bk1.reshape(4, 128).T, bb1.reshape(4, 128).T], axis=1)
    wallP1 = np.ascontiguousarray(
        np.concatenate([pw1p, pwb1], axis=1)).astype(f16)
    assert wallP1.shape[1] == WP1
    wmqs = []
    for q in range(4):
        wmq_ = np.concatenate(
            [pk2[512 * q:512 * (q + 1)].T, pb2[128 * q:128 * (q + 1)].T],
            axis=1)                                   # [512, 640]
        wmqs.append(_pack128(np.ascontiguousarray(wmq_)).astype(f16))

    permm = np.zeros((4, 128, 128), np.float32)
    for m2 in range(4):
        for p in range(128):
            permm[m2, 4 * (p // 4) + m2, p] = 1.0
    permm = permm.transpose(1, 0, 2).reshape(128, 512)
    identm = np.eye(128, dtype=np.float32)
    # selm rows k2 = t2*4 + i2 (t-major, matching D's free layout), padded
    selm = np.zeros((36, 9, 128), np.float32)
    for t in range(9):
        for p in range(128):
            selm[t * 4 + (p % 4), t, p] = 1.0
    selm = np.concatenate(
        [selm.reshape(36, 9 * 128), np.zeros((92, 9 * 128), np.float32)], 0)
    maskm = np.zeros((128, 128), np.float32)
    for p in range(128):
        maskm[p, 4 * (p // 4):4 * (p // 4) + 4] = 1.0
    maskm = np.tile(maskm, (1, 3))
    biasD = np.broadcast_to(
        b2.reshape(512, 4)[:, None, :], (512, 9, 4)).reshape(512, 36)
    biasD = _pack128(biasD)
    biasPK = _pack128(bk2.reshape(512, 4))
    biasPB = bb2.reshape(4, 128).T
    wallP3 = np.ascontiguousarray(np.concatenate(
        [permm, identm, selm, maskm, biasD, biasPK, biasPB],
        axis=1)).astype(f16)
    assert wallP3.shape[1] == WP3

    # w2 quarters (shared): [128, (ib, dydx, k-slice 512)]
    w2qs = []
    for q in range(4):
        w2s = w2[q * 512:(q + 1) * 512]          # [512, 512, 2, 2]
        w2m_ = w2s.transpose(1, 2, 3, 0)         # [512i, 2, 2, 512k]
        w2m_ = (w2m_.reshape(4, 128, 2, 2, 512)
                .transpose(1, 0, 2, 3, 4)
                .reshape(128, 16 * 512))
        w2qs.append(np.ascontiguousarray(w2m_).astype(f16))

    # padded input, per core
    xpad_all = np.pad(pred, ((0, 0), (0, 0), (1, 1), (1, 1)), mode="reflect")
    xpad_all = xpad_all.reshape(N, 512, HWP * HWP).astype(f16)
    st_all = style.transpose(0, 2, 3, 1).reshape(N, NPOS, 512)

    in_maps = []
    for c in range(N):
        xz = np.zeros((512, XPW), f16)
        xz[:, :HWP * HWP] = xpad_all[c]
        xz = xz.reshape(4, 128, XPW)
        # own-sample style [512, 16] -> [128, 4*16]
        st_own = _pack128(np.ascontiguousarray(st_all[c].T))
        wallA = np.concatenate([st_own, w1p, b1t], axis=1).astype(f16)
        assert wallA.shape[1] == WA
        m = {
            "wallA": np.ascontiguousarray(wallA),
            "wallP1": wallP1,
            "wallP3": wallP3,
        }
        for ch in range(4):
            m[f"xp{ch}"] = np.ascontiguousarray(xz[ch])
        for q in range(4):
            m[f"w2q{q}"] = w2qs[q]
            m[f"wmq{q}"] = wmqs[q]
        in_maps.append(m)
    return in_maps


def kernel(**inputs):
    install_profile_shim()
    from concourse.bass_utils import run_bass_kernel_spmd

    nc = _get_nc()
    in_maps = make_in_maps(inputs)
    res = run_bass_kernel_spmd(nc, in_maps, core_ids=list(range(N)))
    outs = [np.asarray(res.results[c]["out"]).reshape(COUT, HW, HW)
            for c in range(N)]
    return np.stack(outs, axis=0).astype(np.float32)

